# revision 59
# baseline (speedup 1.0000x reference)
"""DeepseekV2 MLA prefill attention on 8 NeuronCores (Trainium2, Bass/Tile).

Sharding: token-parallel with zigzag blocks for causal balance. Core c owns
token blocks {c, 15-c} (128 tokens each). Down/up projections and attention
computed per-core for own tokens; K^T is head-sharded then AllGathered
(token-ordered columns by construction); V is token-sharded and AllGathered
(rank-major rows, handled by static slot addressing).

Optimizations over the first working version:
- All weight matrices are pre-arranged on the host so every SBUF slab load is
  a contiguous-row DMA (the naive (a p) c -> p a c gathers were 256B-segment
  scatters that dominated DMA time).
- Phases reordered so the three AllGathers overlap independent compute:
  kv-latent first -> AG(latent) runs under V projection; AG(V) runs under the
  q-latent down-proj + K^T; AG(K^T) runs under the Q up-projection.
- Softmax is single-pass without max subtraction (scores are bounded ~6, exp
  cannot overflow): exp with sum accumulation, unnormalized PV, and the
  1/sum normalization folded into the attnT -> bf16 cast in the out-proj
  phase via a broadcast matmul.
- RoPE rotate-half swaps use a 64x64 permutation matmul instead of
  SBUF-to-SBUF DMAs; V is loaded with one DMA per head; K^T/V shard stores
  are batched.

SPMD constraint: one program for all 8 cores. All per-core variation is
carried by input data (additive masks and 0/1 selectors); the instruction
stream is fully static.
"""
import sys
import json
import os

sys.path.insert(0, "/opt/trn_rl_repo")

import numpy as np
import ml_dtypes

import concourse.bass as bass
import concourse.mybir as mybir
import concourse.tile as tile
from concourse.bass_utils import run_bass_kernel_spmd

F32 = mybir.dt.float32
F32R = mybir.dt.float32r
BF16 = mybir.dt.bfloat16

T = 2048
H = 32
HID = 5120
QL = 1536
KVL = 512
DN = 128
DR = 64
DQK = DN + DR
DV = 128
EPS = 1e-6
SCALING = DQK ** -0.5
NCORES = 8
OWN = 256           # tokens per core
CH = 512            # attention key chunk
NU = 5              # attention units per core
NEG = -1e30

HT = HID // 128     # 40 hidden tiles
QLT = QL // 128     # 12
KVT = KVL // 128    # 4
KVTR = KVT + 1      # + padded rope slab


def _unit_descs(c):
    """Units for core c: [(tile, chunk)] with tile in 'A'(block c)/'B'(block 15-c)."""
    u = [("B", 0), ("B", 1), ("B", 2), ("A", 0)]
    u.append(("A", 1) if c >= 4 else ("B", 3))
    return u


def _vslot(j):
    """rank-major slot index of token block j in the vfull AG layout."""
    return 2 * j if j < 8 else 2 * (15 - j) + 1


def legalize_sync_waits(nc):
    """This container's walrus accepts at most one sync-wait per instruction;
    split extras onto standalone EventSemaphore waits just before (same
    engine; engine streams preserve intra-block order)."""
    m = json.loads(nc.to_json_bytes())
    ctr = [0]

    def fresh():
        ctr[0] += 1
        return f"I-lw-{ctr[0]}"

    for f in m["functions"]:
        for bb in f["blocks"]:
            out = []
            for ins in bb["instructions"]:
                si = ins.get("sync_info")
                waits = (si or {}).get("on_wait") or []
                if len(waits) > 1:
                    for w in waits[:-1]:
                        out.append({
                            "debug": ins.get("debug", 0),
                            "engine": ins["engine"],
                            "ins": [], "outs": [],
                            "name": fresh(),
                            "opcode": "EventSemaphore",
                            "sync_info": {"on_update": [], "on_wait": [w]},
                        })
                    si["on_wait"] = waits[-1:]
                out.append(ins)
            bb["instructions"] = out
    nc.m = mybir.module_from_json_bytes(json.dumps(m).encode())
    return nc


def build_bass(sim_mode=False):
    nc = bass.Bass()
    AL = mybir.AluOpType
    AF = mybir.ActivationFunctionType

    dp = nc.declare_dram_parameter
    hidP_d = dp("hidP", [128, HT * OWN], BF16, isOutput=False)
    cosT_d = dp("cosT", [128, OWN], F32, isOutput=False)
    sinTs_d = dp("sinTs", [128, OWN], F32, isOutput=False)
    wqaP_d = dp("wqaP", [QLT * 128, HT * 128], BF16, isOutput=False)
    # head-pair slabs: [nope_even | nope_odd | rope_even | rope_odd] per lt
    wqbP_d = dp("wqbP", [(H // 2) * 128, QLT * 384], BF16, isOutput=False)
    wkvaP_d = dp("wkvaP", [KVTR * 128, HT * 128], BF16, isOutput=False)
    wkvbnP_d = dp("wkvbnP", [128, KVT * 4 * DN], BF16, isOutput=False)
    wkvbvP_d = dp("wkvbvP", [8 * 128, KVT * CH], BF16, isOutput=False)
    woP_d = dp("woP", [HT * 128, H * 128], BF16, isOutput=False)
    mask5_d = dp("mask5", [128, NU, CH], F32, isOutput=False)
    selA01_d = dp("selA01", [128, NU], F32, isOutput=False)
    selB01_d = dp("selB01", [128, NU], F32, isOutput=False)
    selAu4_d = dp("selAu4", [128, 1], F32, isOutput=False)
    selBu4_d = dp("selBu4", [128, 1], F32, isOutput=False)
    ident_d = dp("ident", [128, 128], BF16, isOutput=False)
    perm128_d = dp("perm128", [128, 128], F32, isOutput=False)
    ones128_d = dp("ones128", [128, 1], F32R, isOutput=False)
    onesrow_d = dp("onesrow", [1, 128], F32, isOutput=False)
    outT_d = dp("outT", [HID, OWN], F32, isOutput=True)

    with tile.TileContext(nc) as tc:
        from contextlib import ExitStack
        st = ExitStack()
        const = st.enter_context(tc.tile_pool(name="const", bufs=1))
        dram = st.enter_context(tc.tile_pool(name="dram", bufs=1, space="DRAM"))

        # ---- constants / tables ----
        ident = const.tile([128, 128], BF16)
        nc.gpsimd.dma_start(ident[:], ident_d[:])
        perm128 = const.tile([128, 128], F32)
        nc.gpsimd.dma_start(perm128[:], perm128_d[:])
        identf = const.tile([128, 128], F32)
        nc.vector.tensor_copy(identf[:], ident[:])
        ones128 = const.tile([128, 1], F32R)
        nc.gpsimd.dma_start(ones128[:], ones128_d[:])
        onesrow = const.tile([1, 128], F32)
        nc.gpsimd.dma_start(onesrow[:], onesrow_d[:])
        mask5 = const.tile([128, NU, CH], F32)
        nc.gpsimd.dma_start(mask5[:], mask5_d[:])
        selA01 = const.tile([128, NU], F32)
        nc.gpsimd.dma_start(selA01[:], selA01_d[:])
        selB01 = const.tile([128, NU], F32)
        nc.gpsimd.dma_start(selB01[:], selB01_d[:])
        selAu4 = const.tile([128, 1], F32)
        nc.gpsimd.dma_start(selAu4[:], selAu4_d[:])
        selBu4 = const.tile([128, 1], F32)
        nc.gpsimd.dma_start(selBu4[:], selBu4_d[:])
        cosT = const.tile([128, OWN], F32)
        nc.gpsimd.dma_start(cosT[:], cosT_d[:])
        sinTs = const.tile([128, OWN], F32)
        nc.gpsimd.dma_start(sinTs[:], sinTs_d[:])
        epsc = const.tile([1, 1], F32)
        nc.vector.memset(epsc[:], EPS)

        # ---- DRAM intermediates / collective buffers ----
        # agin latent rows interleaved (p a) so the post-AG read is contiguous
        # V and K^T collectives are split 4-ways each so attention can start
        # as soon as its first head-group arrives.
        agin = dram.tile([KVL + DR, OWN], BF16)
        agkv = dram.tile([NCORES * (KVL + DR), OWN], BF16, addr_space="Shared")
        vshards = [dram.tile([OWN, 16 * DV], BF16, name=f"vshard{j}")
                   for j in range(2)]
        vfulls = [dram.tile([T, 16 * DV], BF16, addr_space="Shared",
                            name=f"vfull{j}") for j in range(2)]
        ktshards = [dram.tile([2 * DN, T], BF16, name=f"ktshard{j}")
                    for j in range(2)]
        ktfulls = [dram.tile([NCORES * 2 * DN, T], BF16, addr_space="Shared",
                             name=f"ktfull{j}") for j in range(2)]

        # =========== phase B: down projections (transposed) ===========
        latp = st.enter_context(tc.tile_pool(name="latp", bufs=1))
        hidp = st.enter_context(tc.tile_pool(name="hidp", bufs=1))

        hidT = hidp.tile([128, HT, OWN], BF16)
        nc.sync.dma_start(hidT[:], hidP_d.rearrange("p (a t) -> p a t", t=OWN))

        latq_n = latp.tile([128, QLT, OWN], BF16)
        latkv_n = latp.tile([128, KVT, OWN], BF16)

        def down_slab(pools, w_d, lt, dst):
            wsl, rawp, psB, psS = pools
            wslab = wsl.tile([128, HT, 128], BF16, tag="wslab")
            nc.sync.dma_start(
                wslab[:], w_d[128 * lt:128 * (lt + 1), :]
                .rearrange("p (a c) -> p a c", c=128))
            ps = psB.tile([128, OWN], F32, tag="dps")
            for ht in range(HT):
                nc.tensor.matmul(ps[:], wslab[:, ht, :], hidT[:, ht, :],
                                 start=(ht == 0), stop=(ht == HT - 1))
            nc.scalar.copy(dst[:], ps[:])

        def rmsnorm(pools, lat, lat_n, nt, L):
            wsl, rawp, psB, psS = pools
            ssq = psS.tile([1, OWN], F32, tag="ssq")
            for lt in range(nt):
                sq = rawp.tile([128, OWN], F32R, tag="sqscratch", bufs=2)
                nc.vector.tensor_tensor(out=sq[:], in0=lat[:, lt, :],
                                        in1=lat[:, lt, :], op=AL.mult)
                nc.tensor.matmul(ssq[:], ones128[:], sq[:],
                                 start=(lt == 0), stop=(lt == nt - 1))
            f = rawp.tile([1, OWN], F32, tag="fscratch", bufs=2)
            nc.scalar.activation(f[:], ssq[:], AF.Sqrt, bias=epsc[:], scale=1.0 / L)
            fr = rawp.tile([1, OWN], F32, tag="frscratch", bufs=2)
            nc.vector.reciprocal(fr[:], f[:])
            fb = psS.tile([128, OWN], F32, tag="fbcast")
            nc.tensor.matmul(fb[:], onesrow[:], fr[:], start=True, stop=True)
            for lt in range(nt):
                nc.vector.tensor_tensor(out=lat_n[:, lt, :], in0=lat[:, lt, :],
                                        in1=fb[:], op=AL.mult)

        def open_proj_pools(ph, sfx):
            return (
                ph.enter_context(tc.tile_pool(name="wsl" + sfx, bufs=2)),
                ph.enter_context(tc.tile_pool(name="raw" + sfx, bufs=1)),
                ph.enter_context(tc.tile_pool(name="psB" + sfx, bufs=3,
                                              space="PSUM")),
                ph.enter_context(tc.tile_pool(name="psS" + sfx, bufs=1,
                                              space="PSUM")),
            )

        # ===== phase B1: KV latent down-proj + rmsnorm + rope -> AG =====
        ph = ExitStack()
        pools = open_proj_pools(ph, "1")
        rawp, psS = pools[1], pools[3]
        latkv = rawp.tile([128, KVTR, OWN], F32)
        for lt in range(KVTR):
            down_slab(pools, wkvaP_d, lt, latkv[:, lt, :])
        rmsnorm(pools, latkv, latkv_n, KVT, KVL)

        # ---- rope k_pe (deinterleave folded into wkv_a on host) ----
        # swap halves via permutation matmul (rows 0:32 <-> 32:64)
        kpsw = psS.tile([DR, OWN], F32, tag="kpsw", bufs=1)
        nc.tensor.matmul(kpsw[:], perm128[0:DR, 0:DR], latkv[0:DR, KVT, :],
                         start=True, stop=True)
        kpc = rawp.tile([DR, OWN], F32)
        nc.vector.tensor_tensor(out=kpc[:], in0=latkv[0:DR, KVT, :],
                                in1=cosT[0:DR, :], op=AL.mult)
        kpsw2 = rawp.tile([DR, OWN], F32)
        nc.vector.tensor_tensor(out=kpsw2[:], in0=kpsw[:],
                                in1=sinTs[0:DR, :], op=AL.mult)
        kpeR = rawp.tile([DR, OWN], BF16)
        nc.vector.tensor_tensor(out=kpeR[:], in0=kpc[:],
                                in1=kpsw2[:], op=AL.add)

        # assemble AG input: latent rows interleaved (p a), rope rows at 512+
        nc.sync.dma_start(
            agin[0:KVL, :].rearrange("(p a) t -> p a t", a=KVT), latkv_n[:])
        nc.sync.dma_start(agin[KVL:KVL + DR, :], kpeR[:])
        if sim_mode:
            nc.sync.dma_start(agkv[0:KVL + DR, :], agin[:])
        else:
            nc.gpsimd.collective_compute(
                "AllGather", AL.bypass, replica_groups=[list(range(NCORES))],
                ins=[agin.opt()], outs=[agkv.opt()])
        ph.close()

        # ===== phases D+E interleaved: V chunks and K^T heads alternate ====
        # so the 8 AG slices stream v0,kt0,v1,kt1,... and attention can
        # start as soon as (v0, kt0) land.
        ph = ExitStack()
        wkn = ph.enter_context(tc.tile_pool(name="wkn", bufs=1))
        psE = ph.enter_context(tc.tile_pool(name="psE", bufs=2, space="PSUM"))
        eve = ph.enter_context(tc.tile_pool(name="eve", bufs=2))
        agp = ph.enter_context(tc.tile_pool(name="agp", bufs=1))
        phD = ExitStack()
        wv = phD.enter_context(tc.tile_pool(name="wv", bufs=2))
        psD = phD.enter_context(tc.tile_pool(name="psD", bufs=2, space="PSUM"))
        evp = phD.enter_context(tc.tile_pool(name="evp", bufs=3))

        wkns = wkn.tile([128, KVT, 4 * DN], BF16)
        nc.sync.dma_start(
            wkns[:], wkvbnP_d.rearrange("p (a c) -> p a c", c=4 * DN))
        # agkv-dependent loads go on the gpsimd queue: they must wait for
        # the latent AllGather anyway, and on the sync queue they would
        # head-of-line block the weight streams behind them.
        # all 8 latent slabs resident (16 KB/partition)
        slabs = agp.tile([128, NCORES, KVT, OWN], BF16)
        for r in range(NCORES):
            nc.gpsimd.dma_start(
                slabs[:, r, :, :], agkv[(KVL + DR) * r:(KVL + DR) * r + KVL, :]
                .rearrange("(p a) t -> p a t", a=KVT))
        # k_pe^T assembly (token-ordered; duplicated on rows 64:128 so
        # odd heads of a pair can use base partition 64)
        kpeT = const.tile([128, T], BF16)
        for b in range(16):
            rb = min(b, 15 - b)
            colsl = slice(0, 128) if b < 8 else slice(128, 256)
            src_ap = agkv[(KVL + DR) * rb + KVL:(KVL + DR) * rb + KVL + DR, colsl]
            nc.gpsimd.dma_start(kpeT[0:DR, 128 * b:128 * (b + 1)], src_ap)
            nc.gpsimd.dma_start(kpeT[DR:128, 128 * b:128 * (b + 1)], src_ap)

        def d_chunk(j):
            for vc2 in range(4):        # 4 chunks of 512 v-columns each
                vc = 4 * j + vc2
                wvs = wv.tile([128, KVT, CH], BF16, tag="wvs")
                nc.sync.dma_start(
                    wvs[:], wkvbvP_d[128 * vc:128 * (vc + 1), :]
                    .rearrange("p (a c) -> p a c", c=CH))
                ev = evp.tile([128, 2, CH], BF16, tag="vev")
                for tt in range(2):     # 2 token tiles of 128
                    ps = psD.tile([128, CH], F32, tag="vps")
                    for lt in range(KVT):
                        nc.tensor.matmul(
                            ps[:], latkv_n[:, lt, 128 * tt:128 * (tt + 1)],
                            wvs[:, lt, :], start=(lt == 0),
                            stop=(lt == KVT - 1))
                    nc.scalar.copy(ev[:, tt, :], ps[:])
                nc.sync.dma_start(
                    vshards[j][:, CH * vc2:CH * (vc2 + 1)]
                    .rearrange("(tt p) c -> p tt c", p=128), ev[:])
            if sim_mode:
                nc.sync.dma_start(vfulls[j][0:OWN, :], vshards[j][:])
            else:
                nc.gpsimd.collective_compute(
                    "AllGather", AL.bypass,
                    replica_groups=[list(range(NCORES))],
                    ins=[vshards[j].opt()], outs=[vfulls[j].opt()])

        def e_head(hl):
            evA = eve.tile([128, NCORES, 128], BF16, tag="ktevA")
            evB = eve.tile([128, NCORES, 128], BF16, tag="ktevB")
            for r in range(NCORES):
                ps = psE.tile([128, OWN], F32, tag="ktps")
                for lt in range(KVT):
                    nc.tensor.matmul(ps[:], wkns[:, lt, DN * hl:DN * (hl + 1)],
                                     slabs[:, r, lt, :],
                                     start=(lt == 0), stop=(lt == KVT - 1))
                # rank r owns token blocks r (cols 0:128) and 15-r (128:256)
                nc.scalar.copy(evA[:, r, :], ps[:, 0:128])
                nc.scalar.copy(evB[:, 7 - r, :], ps[:, 128:256])
            half = hl // 2
            row0 = DN * (hl % 2)
            nc.sync.dma_start(ktshards[half][row0:row0 + DN, 0:1024], evA[:])
            nc.sync.dma_start(ktshards[half][row0:row0 + DN, 1024:2048], evB[:])
            if hl % 2 == 1:
                if sim_mode:
                    nc.sync.dma_start(ktfulls[half][0:2 * DN, :],
                                      ktshards[half][:])
                else:
                    nc.gpsimd.collective_compute(
                        "AllGather", AL.bypass,
                        replica_groups=[list(range(NCORES))],
                        ins=[ktshards[half].opt()], outs=[ktfulls[half].opt()])

        for i in range(2):
            d_chunk(i)
        phD.close()

        # q latent down-proj between D and E: E's inputs need the latent
        # AllGather, so E compute would stall the in-order tensor queue if
        # emitted first; B2 fills that window.
        ph2 = ExitStack()
        pools = open_proj_pools(ph2, "2")
        latq = pools[1].tile([128, QLT, OWN], F32)
        for lt in range(QLT):
            down_slab(pools, wqaP_d, lt, latq[:, lt, :])
        rmsnorm(pools, latq, latq_n, QLT, QL)
        ph2.close()

        for i in range(4):
            e_head(i)
        ph.close()

        # ===== phase F: Q up-projection + rope (head pairs) ===========
        # pair slab columns per lt: [nope_even(128) | nope_odd(128) |
        # rope_even(64) | rope_odd(64)]; rope halves share one stationary.
        qp_pool = st.enter_context(tc.tile_pool(name="qp", bufs=1))
        qTn = qp_pool.tile([128, H, OWN], BF16)
        qTp = qp_pool.tile([128, H // 2, OWN], BF16)
        attnB = qp_pool.tile([128, H, OWN], BF16)
        recips = qp_pool.tile([128, H, 2], F32)

        ph = ExitStack()
        wqb = ph.enter_context(tc.tile_pool(name="wqb", bufs=3))
        psF = ph.enter_context(tc.tile_pool(name="psF", bufs=2, space="PSUM"))
        rp = ph.enter_context(tc.tile_pool(name="rp", bufs=3))

        for pr in range(H // 2):
            ws = wqb.tile([128, QLT, 384], BF16, tag="wqbs")
            nc.sync.dma_start(
                ws[:], wqbP_d[128 * pr:128 * (pr + 1), :]
                .rearrange("p (a c) -> p a c", c=384))
            psnE = psF.tile([128, OWN], F32, tag="qnE")
            psnO = psF.tile([128, OWN], F32, tag="qnO")
            psp = psF.tile([128, OWN], F32, tag="qpp")
            for lt in range(QLT):
                nc.tensor.matmul(psnE[:], ws[:, lt, 0:128], latq_n[:, lt, :],
                                 start=(lt == 0), stop=(lt == QLT - 1))
            for lt in range(QLT):
                nc.tensor.matmul(psnO[:], ws[:, lt, 128:256], latq_n[:, lt, :],
                                 start=(lt == 0), stop=(lt == QLT - 1))
            for lt in range(QLT):
                nc.tensor.matmul(psp[:], ws[:, lt, 256:384],
                                 latq_n[:, lt, :],
                                 start=(lt == 0), stop=(lt == QLT - 1))
            nc.scalar.copy(qTn[:, 2 * pr, :], psnE[:])
            nc.scalar.copy(qTn[:, 2 * pr + 1, :], psnO[:])
            praw = rp.tile([128, OWN], F32, tag="praw")
            nc.scalar.copy(praw[:], psp[:])
            psw = psF.tile([128, OWN], F32, tag="psw")
            nc.tensor.matmul(psw[:], perm128[:], praw[:], start=True, stop=True)
            pc = rp.tile([128, OWN], F32, tag="pc")
            nc.vector.tensor_tensor(out=pc[:], in0=praw[:],
                                    in1=cosT[:], op=AL.mult)
            psw2 = rp.tile([128, OWN], F32, tag="psw2")
            nc.vector.tensor_tensor(out=psw2[:], in0=psw[:],
                                    in1=sinTs[:], op=AL.mult)
            nc.vector.tensor_tensor(out=qTp[:, pr, :], in0=pc[:],
                                    in1=psw2[:], op=AL.add)
        ph.close()

        # wo slabs prefetch on the (otherwise idle) gpsimd DMA queue while
        # attention runs; the pool lives outside G's pools so its SBUF does
        # not alias G tiles.
        phw = ExitStack()
        wop = phw.enter_context(tc.tile_pool(name="wop", bufs=8))
        wo_tiles = {}

        def wo_load(oc):
            ws = wop.tile([128, H, 128], BF16, tag="wos")
            nc.gpsimd.dma_start(
                ws[:], woP_d[128 * oc:128 * (oc + 1), :]
                .rearrange("p (a c) -> p a c", c=128))
            wo_tiles[oc] = ws

        # =========== phase G: attention (single-pass softmax, no max) ======
        # Software-pipelined: head h+1's scores are issued before head h's
        # PV block so the PE stream never waits on the softmax chain.
        ph = ExitStack()
        ap = ph.enter_context(tc.tile_pool(name="ap", bufs=2))
        sp_pool = ph.enter_context(tc.tile_pool(name="spp", bufs=2))
        stt = ph.enter_context(tc.tile_pool(name="stt", bufs=2))
        abuf = ph.enter_context(tc.tile_pool(name="abuf", bufs=2))
        psG = ph.enter_context(tc.tile_pool(name="psG", bufs=2, space="PSUM"))
        psT = ph.enter_context(tc.tile_pool(name="psT", bufs=2, space="PSUM"))
        psV = ph.enter_context(tc.tile_pool(name="psV", bufs=2, space="PSUM"))
        psN = ph.enter_context(tc.tile_pool(name="psN", bufs=1, space="PSUM"))

        # unit static params: (qsel, koff) ; u4 handled as two variants
        UQ = [1, 1, 1, 0]          # u0-2: tile B (q cols 128:256), u3: tile A
        UK = [0, CH, 2 * CH, 0]
        U4A = (0, CH)              # qsel A, keys 512:1024
        U4B = (1, 3 * CH)          # qsel B, keys 1536:2048
        # V slot indices (rank-major AG layout) per unit
        UVB = [[_vslot(j) for j in blocks] for blocks in
               [[0, 1, 2, 3], [4, 5, 6, 7], [8, 9, 10, 11], [0, 1, 2, 3]]]
        U4AV = [_vslot(j) for j in [4, 5, 6, 7]]
        U4BV = [_vslot(j) for j in [12, 13, 14, 15]]

        Pbs = {}
        sumus = {}

        def g_pass1(h):
            kt = ap.tile([128, T], BF16, tag="kt")
            row0 = 2 * DN * (h // 4) + DN * (h % 2)
            nc.sync.dma_start(
                kt[:], ktfulls[(h % 4) // 2][row0:row0 + DN, :])
            Pb = sp_pool.tile([128, NU, CH], BF16, tag="Pb")
            sumu = stt.tile([128, NU], F32, tag="sumu")
            Pbs[h] = Pb
            sumus[h] = sumu
            pb = 64 * (h % 2)
            pr = h // 2

            def scores(qsel, koff, psname):
                ps = psG.tile([128, CH], F32, tag="sps", name=psname)
                q0 = 128 * qsel
                nc.tensor.matmul(ps[:], qTn[:, h, q0:q0 + 128],
                                 kt[:, koff:koff + CH], start=True, stop=False)
                nc.tensor.matmul(ps[:], qTp[pb:pb + DR, pr, q0:q0 + 128],
                                 kpeT[pb:pb + DR, koff:koff + CH],
                                 start=False, stop=True)
                return ps

            def expu(u, src):
                nc.scalar.activation(Pb[:, u, :], src, AF.Exp,
                                     scale=SCALING,
                                     accum_out=sumu[:, u:u + 1])

            # u0, u1: fully causal-visible; exp straight from PSUM
            for u in (0, 1):
                ps = scores(UQ[u], UK[u], f"s{h}_{u}")
                expu(u, ps[:])
            # u2, u3: mask-add then exp
            for u in (2, 3):
                ps = scores(UQ[u], UK[u], f"s{h}_{u}")
                S = sp_pool.tile([128, CH], F32, tag="Smask", bufs=3)
                nc.vector.tensor_tensor(out=S[:], in0=ps[:],
                                        in1=mask5[:, u, :], op=AL.add)
                expu(u, S[:])
            # u4: select variant, mask, exp
            ps4a = scores(*U4A, f"s{h}_4a")
            ps4b = scores(*U4B, f"s{h}_4b")
            S4 = sp_pool.tile([128, CH], F32, tag="Smask", bufs=3)
            nc.vector.scalar_tensor_tensor(
                out=S4[:], in0=ps4a[:], scalar=selAu4[:],
                in1=mask5[:, 4, :], op0=AL.mult, op1=AL.add)
            nc.vector.scalar_tensor_tensor(
                out=S4[:], in0=ps4b[:], scalar=selBu4[:],
                in1=S4[:], op0=AL.mult, op1=AL.add)
            expu(4, S4[:])

            # per-tile softmax denominators (reciprocals, per token row)
            sumu2 = stt.tile([128, 2], F32, tag="sumu2")
            tA = stt.tile([128, NU], F32, tag="tA")
            nc.vector.tensor_tensor(out=tA[:], in0=sumu[:], in1=selA01[:],
                                    op=AL.mult)
            nc.vector.tensor_reduce(sumu2[:, 0:1], tA[:],
                                    axis=mybir.AxisListType.X, op=AL.add)
            tB = stt.tile([128, NU], F32, tag="tB")
            nc.vector.tensor_tensor(out=tB[:], in0=sumu[:], in1=selB01[:],
                                    op=AL.mult)
            nc.vector.tensor_reduce(sumu2[:, 1:2], tB[:],
                                    axis=mybir.AxisListType.X, op=AL.add)
            nc.vector.reciprocal(recips[:, h, :], sumu2[:])

        def g_pass2(h):
            vh = ap.tile([128, 16, DV], BF16, tag="vh")
            nc.sync.dma_start(
                vh[:], vfulls[h // 16].rearrange("(b p) d -> p b d", p=128)
                [:, :, DV * (h % 16):DV * (h % 16 + 1)])
            Pb = Pbs.pop(h)
            sumus.pop(h)

            # transpose all 4 key-blocks of a unit into one PSUM bank,
            # evict with a single 512-wide copy
            def ptT_unit(u, engine):
                tp = psT.tile([128, 4, 128], BF16, tag="tp")
                for kb in range(4):
                    nc.tensor.transpose(tp[:, kb, :],
                                        Pb[:, u, 128 * kb:128 * (kb + 1)],
                                        ident[:])
                ptT = stt.tile([128, 4, 128], BF16, tag="ptT", bufs=3)
                if engine == 0:
                    nc.vector.tensor_copy(ptT[:], tp[:])
                else:
                    nc.scalar.copy(ptT[:], tp[:])
                return ptT

            att = abuf.tile([128, OWN], F32, tag="att")

            # units 0-2 accumulate into one PSUM tile (B half)
            psBh = psV.tile([128, 128], F32, tag="pvps", name=f"pvB{h}")
            for u in (0, 1, 2):
                ptT = ptT_unit(u, u % 2)
                for kb in range(4):
                    nc.tensor.matmul(psBh[:], vh[:, UVB[u][kb], :],
                                     ptT[:, kb, :],
                                     start=(u == 0 and kb == 0),
                                     stop=(u == 2 and kb == 3))
            nc.vector.tensor_copy(att[:, 128:256], psBh[:])
            # unit 3 (A half)
            psAh = psV.tile([128, 128], F32, tag="pvps", name=f"pvA{h}")
            ptT = ptT_unit(3, 1)
            for kb in range(4):
                nc.tensor.matmul(psAh[:], vh[:, UVB[3][kb], :], ptT[:, kb, :],
                                 start=(kb == 0), stop=(kb == 3))
            nc.vector.tensor_copy(att[:, 0:128], psAh[:])
            # u4: same P^T, two V variants, masked adds
            ps4av = psV.tile([128, 128], F32, tag="pvps", name=f"pv{h}_4a")
            ps4bv = psV.tile([128, 128], F32, tag="pvps", name=f"pv{h}_4b")
            ptT = ptT_unit(4, 0)
            for kb in range(4):
                nc.tensor.matmul(ps4av[:], vh[:, U4AV[kb], :], ptT[:, kb, :],
                                 start=(kb == 0), stop=(kb == 3))
                nc.tensor.matmul(ps4bv[:], vh[:, U4BV[kb], :], ptT[:, kb, :],
                                 start=(kb == 0), stop=(kb == 3))
            nc.vector.scalar_tensor_tensor(
                out=att[:, 0:128], in0=ps4av[:], scalar=selAu4[:],
                in1=att[:, 0:128], op0=AL.mult, op1=AL.add)
            nc.vector.scalar_tensor_tensor(
                out=att[:, 128:256], in0=ps4bv[:], scalar=selBu4[:],
                in1=att[:, 128:256], op0=AL.mult, op1=AL.add)

            # normalize: transpose per-token reciprocals into one row,
            # broadcast to all partitions, multiply
            rT = psN.tile([1, OWN], F32, tag="rT", bufs=1)
            nc.tensor.matmul(rT[0:1, 0:128], recips[:, h, 0:1], identf[:],
                             start=True, stop=True)
            nc.tensor.matmul(rT[0:1, 128:256], recips[:, h, 1:2], identf[:],
                             start=True, stop=True)
            recT = stt.tile([1, OWN], F32, tag="recT", bufs=2)
            nc.scalar.copy(recT[:], rT[:])
            nb = psN.tile([128, OWN], F32, tag="nb", bufs=1)
            nc.tensor.matmul(nb[:], onesrow[:], recT[:], start=True, stop=True)
            nc.vector.tensor_tensor(out=attnB[:, h, :], in0=nb[:],
                                    in1=att[:], op=AL.mult)

        # process heads in AG-arrival order: K^T slice i carries heads 4r+i
        HORDER = [4 * r + i for i in range(4) for r in range(NCORES)]
        for oc in range(8):
            wo_load(oc)
        g_pass1(HORDER[0])
        for idx, h in enumerate(HORDER):
            if idx + 1 < H:
                g_pass1(HORDER[idx + 1])
            g_pass2(h)
        ph.close()

        # =========== phase H: out projection ===========
        ph = ExitStack()
        psH = ph.enter_context(tc.tile_pool(name="psH", bufs=4, space="PSUM"))
        oev = ph.enter_context(tc.tile_pool(name="oev", bufs=3))
        for oc in range(HT):
            if oc + 8 < HT:
                wo_load(oc + 8)
            ws = wo_tiles.pop(oc)
            ps = psH.tile([128, OWN], F32, tag="ops")
            for ct in range(H):
                nc.tensor.matmul(ps[:], ws[:, ct, :], attnB[:, ct, :],
                                 start=(ct == 0), stop=(ct == H - 1))
            ev = oev.tile([128, OWN], F32, tag="oev")
            nc.scalar.copy(ev[:], ps[:])
            nc.sync.dma_start(outT_d[128 * oc:128 * (oc + 1), :], ev[:])
        ph.close()
        phw.close()
        st.close()

    nc.finalize()
    legalize_sync_waits(nc)
    return nc


_DEINT = np.array([2 * r if r < 32 else 2 * r - 63 for r in range(DR)])


def _slabs(w, rows_per_slab=128):
    """[R, C] -> [R/128 * 128, (R_tiles) * C] contiguous slab layout:
    out[s*128 + p, a*C + c] = w[a*128 + p, s*C_slab + c] per slab s of cols."""
    raise NotImplementedError


def _pa(w):
    """[A*128, C] -> [128, A*C]: row-block-major to partition-major."""
    A = w.shape[0] // 128
    return w.reshape(A, 128, w.shape[1]).transpose(1, 0, 2).reshape(128, -1)


def _host_prep(inputs):
    f32 = np.float32
    bf16 = ml_dtypes.bfloat16
    hs = np.asarray(inputs["hidden_states"], f32)
    cos = np.asarray(inputs["cos"], f32).reshape(T, DR)
    sin = np.asarray(inputs["sin"], f32).reshape(T, DR)
    wq_a = np.asarray(inputs["wq_a"], f32)
    q_ln = np.asarray(inputs["q_a_ln_w"], f32)
    wq_b = np.asarray(inputs["wq_b"], f32)
    wkv_a = np.asarray(inputs["wkv_a"], f32)
    kv_ln = np.asarray(inputs["kv_a_ln_w"], f32)
    wkv_b = np.asarray(inputs["wkv_b"], f32)
    wo = np.asarray(inputs["wo"], f32)

    # fold ln weights into up-projections
    wq_b = wq_b * q_ln[:, None]
    wkv_b = wkv_b * kv_ln[:, None]

    # deinterleave fold: q_pe columns of wq_b, k_pe columns of wkv_a
    wqbp = wq_b.copy()
    for h in range(H):
        pe = wq_b[:, h * DQK + DN:h * DQK + DQK]
        wqbp[:, h * DQK + DN:h * DQK + DQK] = pe[:, _DEINT]
    wkvap = np.zeros((HID, KVTR * 128), f32)
    wkvap[:, 0:KVL] = wkv_a[:, 0:KVL]
    wkvap[:, KVL:KVL + DR] = wkv_a[:, KVL:][:, _DEINT]

    # split wkv_b into nope / v column groups (head-major)
    wkvbn = np.concatenate(
        [wkv_b[:, h * 256:h * 256 + DN] for h in range(H)], axis=1)  # [512,4096]
    wkvbv = np.concatenate(
        [wkv_b[:, h * 256 + DN:h * 256 + 256] for h in range(H)], axis=1)

    # ---- contiguous slab layouts (one DMA per slab, contiguous rows) ----
    # wqaP: slab lt rows -> [128, 40*128] from wq_a cols [128lt:128lt+128]
    wqaP = np.concatenate(
        [_pa(wq_a[:, 128 * lt:128 * (lt + 1)]) for lt in range(QLT)], axis=0)
    wkvaP = np.concatenate(
        [_pa(wkvap[:, 128 * lt:128 * (lt + 1)]) for lt in range(KVTR)], axis=0)
    # head-pair slabs: [nope_even | nope_odd | rope_even | rope_odd]
    wqbP = np.concatenate(
        [_pa(np.concatenate([
            wqbp[:, DQK * (2 * pr):DQK * (2 * pr) + DN],
            wqbp[:, DQK * (2 * pr + 1):DQK * (2 * pr + 1) + DN],
            wqbp[:, DQK * (2 * pr) + DN:DQK * (2 * pr + 1)],
            wqbp[:, DQK * (2 * pr + 1) + DN:DQK * (2 * pr + 2)],
        ], axis=1)) for pr in range(H // 2)], axis=0)
    wkvbvP = np.concatenate(
        [_pa(wkvbv[:, CH * vc:CH * (vc + 1)]) for vc in range(8)], axis=0)
    woP = np.concatenate(
        [_pa(wo[:, 128 * oc:128 * (oc + 1)]) for oc in range(HT)], axis=0)

    cosTf = np.ascontiguousarray(cos.T)          # [64, 2048]
    sinTf = np.ascontiguousarray(sin.T)
    sinTs_f = sinTf.copy()
    sinTs_f[0:32] = -sinTf[0:32]

    ident = np.eye(128, dtype=bf16)
    perm128 = np.zeros((128, 128), f32)
    for o in range(DR):
        perm128[(o + 32) % DR, o] = 1.0
        perm128[DR + (o + 32) % DR, DR + o] = 1.0
    ones128 = np.ones((128, 1), f32)
    onesrow = np.ones((1, 128), f32)

    wqaP_bf = wqaP.astype(bf16)
    wqbP_bf = wqbP.astype(bf16)
    wkvaP_bf = wkvaP.astype(bf16)
    wkvbvP_bf = wkvbvP.astype(bf16)
    woP_bf = woP.astype(bf16)
    wkvbn_bf = wkvbn.astype(bf16)

    in_maps = []
    for c in range(NCORES):
        bA, bB = c, 15 - c
        own = np.r_[np.arange(128 * bA, 128 * bA + 128),
                    np.arange(128 * bB, 128 * bB + 128)]
        units = _unit_descs(c)
        mask5 = np.zeros((128, NU, CH), f32)
        selA01 = np.zeros((128, NU), f32)
        selB01 = np.zeros((128, NU), f32)
        for u, (tl, j) in enumerate(units):
            b = bA if tl == "A" else bB
            qtok = 128 * b + np.arange(128)[:, None]
            ktok = CH * j + np.arange(CH)[None, :]
            mask5[:, u, :] = np.where(ktok <= qtok, 0.0, NEG)
            (selA01 if tl == "A" else selB01)[:, u] = 1.0
        selAu4 = np.full((128, 1), 1.0 if c >= 4 else 0.0, f32)
        selBu4 = np.full((128, 1), 0.0 if c >= 4 else 1.0, f32)

        hidT = np.ascontiguousarray(hs[own].T)   # [5120, 256]
        hidP = _pa(hidT)                         # [128, 40*256]

        # this core's 4 nope heads: [512, 512] -> [128, 4*512]
        wkvbnP = _pa(wkvbn_bf[:, 4 * DN * c:4 * DN * (c + 1)].astype(f32)).astype(bf16)

        in_maps.append({
            "hidP": hidP.astype(bf16),
            "cosT": np.ascontiguousarray(np.vstack([cosTf[:, own]] * 2)),
            "sinTs": np.ascontiguousarray(np.vstack([sinTs_f[:, own]] * 2)),
            "wqaP": wqaP_bf,
            "wqbP": wqbP_bf,
            "wkvaP": wkvaP_bf,
            "wkvbnP": wkvbnP,
            "wkvbvP": wkvbvP_bf,
            "woP": woP_bf,
            "mask5": mask5,
            "selA01": selA01, "selB01": selB01,
            "selAu4": selAu4, "selBu4": selBu4,
            "ident": ident, "perm128": perm128,
            "ones128": ones128, "onesrow": onesrow,
        })
    return in_maps


_NC_CACHE = None


def _get_nc():
    global _NC_CACHE
    if _NC_CACHE is None:
        _NC_CACHE = build_bass()
    return _NC_CACHE


def run(inputs, trace=False):
    nc = _get_nc()
    in_maps = _host_prep(inputs)
    res = run_bass_kernel_spmd(nc, in_maps, list(range(NCORES)), trace=trace)
    out = np.empty((T, HID), np.float32)
    for c in range(NCORES):
        oT = res.results[c]["outT"]
        out[128 * c:128 * (c + 1)] = oT[:, 0:128].T
        out[128 * (15 - c):128 * (16 - c)] = oT[:, 128:256].T
    return out, res


def kernel(**inputs):
    out, _ = run(inputs, trace=False)
    return out


# revision 62
# speedup vs baseline: 1.0264x; 1.0264x over previous
"""DeepseekV2 MLA prefill attention on 8 NeuronCores (Trainium2, Bass/Tile).

Sharding: token-parallel with zigzag blocks for causal balance. Core c owns
token blocks {c, 15-c} (128 tokens each). Down/up projections and attention
computed per-core for own tokens; K^T is head-sharded then AllGathered
(token-ordered columns by construction); V is token-sharded and AllGathered
(rank-major rows, handled by static slot addressing).

Optimizations over the first working version:
- All weight matrices are pre-arranged on the host so every SBUF slab load is
  a contiguous-row DMA (the naive (a p) c -> p a c gathers were 256B-segment
  scatters that dominated DMA time).
- Phases reordered so the three AllGathers overlap independent compute:
  kv-latent first -> AG(latent) runs under V projection; AG(V) runs under the
  q-latent down-proj + K^T; AG(K^T) runs under the Q up-projection.
- Softmax is single-pass without max subtraction (scores are bounded ~6, exp
  cannot overflow): exp with sum accumulation, unnormalized PV, and the
  1/sum normalization folded into the attnT -> bf16 cast in the out-proj
  phase via a broadcast matmul.
- RoPE rotate-half swaps use a 64x64 permutation matmul instead of
  SBUF-to-SBUF DMAs; V is loaded with one DMA per head; K^T/V shard stores
  are batched.

SPMD constraint: one program for all 8 cores. All per-core variation is
carried by input data (additive masks and 0/1 selectors); the instruction
stream is fully static.
"""
import sys
import json
import os

sys.path.insert(0, "/opt/trn_rl_repo")

import numpy as np
import ml_dtypes

import concourse.bass as bass
import concourse.mybir as mybir
import concourse.tile as tile
from concourse.bass_utils import run_bass_kernel_spmd

F32 = mybir.dt.float32
F32R = mybir.dt.float32r
BF16 = mybir.dt.bfloat16

T = 2048
H = 32
HID = 5120
QL = 1536
KVL = 512
DN = 128
DR = 64
DQK = DN + DR
DV = 128
EPS = 1e-6
SCALING = DQK ** -0.5
NCORES = 8
OWN = 256           # tokens per core
CH = 512            # attention key chunk
NU = 5              # attention units per core
NEG = -1e30

HT = HID // 128     # 40 hidden tiles
QLT = QL // 128     # 12
KVT = KVL // 128    # 4
KVTR = KVT + 1      # + padded rope slab


def _unit_descs(c):
    """Units for core c: [(tile, chunk)] with tile in 'A'(block c)/'B'(block 15-c)."""
    u = [("B", 0), ("B", 1), ("B", 2), ("A", 0)]
    u.append(("A", 1) if c >= 4 else ("B", 3))
    return u


def _vslot(j):
    """rank-major slot index of token block j in the vfull AG layout."""
    return 2 * j if j < 8 else 2 * (15 - j) + 1


def legalize_sync_waits(nc):
    """This container's walrus accepts at most one sync-wait per instruction;
    split extras onto standalone EventSemaphore waits just before (same
    engine; engine streams preserve intra-block order)."""
    m = json.loads(nc.to_json_bytes())
    ctr = [0]

    def fresh():
        ctr[0] += 1
        return f"I-lw-{ctr[0]}"

    for f in m["functions"]:
        for bb in f["blocks"]:
            out = []
            for ins in bb["instructions"]:
                si = ins.get("sync_info")
                waits = (si or {}).get("on_wait") or []
                if len(waits) > 1:
                    for w in waits[:-1]:
                        out.append({
                            "debug": ins.get("debug", 0),
                            "engine": ins["engine"],
                            "ins": [], "outs": [],
                            "name": fresh(),
                            "opcode": "EventSemaphore",
                            "sync_info": {"on_update": [], "on_wait": [w]},
                        })
                    si["on_wait"] = waits[-1:]
                out.append(ins)
            bb["instructions"] = out
    nc.m = mybir.module_from_json_bytes(json.dumps(m).encode())
    return nc


def build_bass(sim_mode=False):
    nc = bass.Bass()
    AL = mybir.AluOpType
    AF = mybir.ActivationFunctionType

    dp = nc.declare_dram_parameter
    hidP_d = dp("hidP", [128, HT * OWN], BF16, isOutput=False)
    cosT_d = dp("cosT", [128, OWN], F32, isOutput=False)
    sinTs_d = dp("sinTs", [128, OWN], F32, isOutput=False)
    wqaP_d = dp("wqaP", [QLT * 128, HT * 128], BF16, isOutput=False)
    # head-pair slabs: [nope_even | nope_odd | rope_even | rope_odd] per lt
    wqbP_d = dp("wqbP", [(H // 2) * 128, QLT * 384], BF16, isOutput=False)
    wkvaP_d = dp("wkvaP", [KVTR * 128, HT * 128], BF16, isOutput=False)
    wkvbnP_d = dp("wkvbnP", [128, KVT * 4 * DN], BF16, isOutput=False)
    wkvbvP_d = dp("wkvbvP", [8 * 128, KVT * CH], BF16, isOutput=False)
    woP_d = dp("woP", [HT * 128, H * 128], BF16, isOutput=False)
    mask5_d = dp("mask5", [128, NU, CH], F32, isOutput=False)
    selA01_d = dp("selA01", [128, NU], F32, isOutput=False)
    selB01_d = dp("selB01", [128, NU], F32, isOutput=False)
    selAu4_d = dp("selAu4", [128, 1], F32, isOutput=False)
    selBu4_d = dp("selBu4", [128, 1], F32, isOutput=False)
    ident_d = dp("ident", [128, 128], BF16, isOutput=False)
    perm128_d = dp("perm128", [128, 128], F32, isOutput=False)
    ones128_d = dp("ones128", [128, 1], F32R, isOutput=False)
    onesrow_d = dp("onesrow", [1, 128], F32, isOutput=False)
    outT_d = dp("outT", [HID, OWN], F32, isOutput=True)

    with tile.TileContext(nc) as tc:
        from contextlib import ExitStack
        st = ExitStack()
        const = st.enter_context(tc.tile_pool(name="const", bufs=1))
        dram = st.enter_context(tc.tile_pool(name="dram", bufs=1, space="DRAM"))

        # ---- constants / tables ----
        ident = const.tile([128, 128], BF16)
        nc.gpsimd.dma_start(ident[:], ident_d[:])
        perm128 = const.tile([128, 128], F32)
        nc.gpsimd.dma_start(perm128[:], perm128_d[:])
        identf = const.tile([128, 128], F32)
        nc.vector.tensor_copy(identf[:], ident[:])
        ones128 = const.tile([128, 1], F32R)
        nc.gpsimd.dma_start(ones128[:], ones128_d[:])
        onesrow = const.tile([1, 128], F32)
        nc.gpsimd.dma_start(onesrow[:], onesrow_d[:])
        mask5 = const.tile([128, NU, CH], F32)
        nc.gpsimd.dma_start(mask5[:], mask5_d[:])
        selA01 = const.tile([128, NU], F32)
        nc.gpsimd.dma_start(selA01[:], selA01_d[:])
        selB01 = const.tile([128, NU], F32)
        nc.gpsimd.dma_start(selB01[:], selB01_d[:])
        selAu4 = const.tile([128, 1], F32)
        nc.gpsimd.dma_start(selAu4[:], selAu4_d[:])
        selBu4 = const.tile([128, 1], F32)
        nc.gpsimd.dma_start(selBu4[:], selBu4_d[:])
        cosT = const.tile([128, OWN], F32)
        nc.gpsimd.dma_start(cosT[:], cosT_d[:])
        sinTs = const.tile([128, OWN], F32)
        nc.gpsimd.dma_start(sinTs[:], sinTs_d[:])
        epsc = const.tile([1, 1], F32)
        nc.vector.memset(epsc[:], EPS)

        # ---- DRAM intermediates / collective buffers ----
        # agin latent rows interleaved (p a) so the post-AG read is contiguous
        # V and K^T collectives are split 4-ways each so attention can start
        # as soon as its first head-group arrives.
        agin = dram.tile([KVL + DR, OWN], BF16)
        agkv = dram.tile([NCORES * (KVL + DR), OWN], BF16, addr_space="Shared")
        vshards = [dram.tile([OWN, 16 * DV], BF16, name=f"vshard{j}")
                   for j in range(2)]
        vfulls = [dram.tile([T, 16 * DV], BF16, addr_space="Shared",
                            name=f"vfull{j}") for j in range(2)]
        ktshards = [dram.tile([2 * DN, T], BF16, name=f"ktshard{j}")
                    for j in range(2)]
        ktfulls = [dram.tile([NCORES * 2 * DN, T], BF16, addr_space="Shared",
                             name=f"ktfull{j}") for j in range(2)]

        # =========== phase B: down projections (transposed) ===========
        latp = st.enter_context(tc.tile_pool(name="latp", bufs=1))
        hidp = st.enter_context(tc.tile_pool(name="hidp", bufs=1))

        hidT = hidp.tile([128, HT, OWN], BF16)
        nc.sync.dma_start(hidT[:], hidP_d.rearrange("p (a t) -> p a t", t=OWN))

        latq_n = latp.tile([128, QLT, OWN], BF16)
        latkv_n = latp.tile([128, KVT, OWN], BF16)

        def down_slab(pools, w_d, lt, dst):
            wsl, rawp, psB, psS = pools
            wslab = wsl.tile([128, HT, 128], BF16, tag="wslab")
            nc.sync.dma_start(
                wslab[:], w_d[128 * lt:128 * (lt + 1), :]
                .rearrange("p (a c) -> p a c", c=128))
            ps = psB.tile([128, OWN], F32, tag="dps")
            for ht in range(HT):
                nc.tensor.matmul(ps[:], wslab[:, ht, :], hidT[:, ht, :],
                                 start=(ht == 0), stop=(ht == HT - 1))
            nc.scalar.copy(dst[:], ps[:])

        def rmsnorm(pools, lat, lat_n, nt, L):
            wsl, rawp, psB, psS = pools
            ssq = psS.tile([1, OWN], F32, tag="ssq")
            for lt in range(nt):
                sq = rawp.tile([128, OWN], F32R, tag="sqscratch", bufs=2)
                nc.vector.tensor_tensor(out=sq[:], in0=lat[:, lt, :],
                                        in1=lat[:, lt, :], op=AL.mult)
                nc.tensor.matmul(ssq[:], ones128[:], sq[:],
                                 start=(lt == 0), stop=(lt == nt - 1))
            f = rawp.tile([1, OWN], F32, tag="fscratch", bufs=2)
            nc.scalar.activation(f[:], ssq[:], AF.Sqrt, bias=epsc[:], scale=1.0 / L)
            fr = rawp.tile([1, OWN], F32, tag="frscratch", bufs=2)
            nc.vector.reciprocal(fr[:], f[:])
            fb = psS.tile([128, OWN], F32, tag="fbcast")
            nc.tensor.matmul(fb[:], onesrow[:], fr[:], start=True, stop=True)
            for lt in range(nt):
                nc.vector.tensor_tensor(out=lat_n[:, lt, :], in0=lat[:, lt, :],
                                        in1=fb[:], op=AL.mult)

        def open_proj_pools(ph, sfx):
            return (
                ph.enter_context(tc.tile_pool(name="wsl" + sfx, bufs=2)),
                ph.enter_context(tc.tile_pool(name="raw" + sfx, bufs=1)),
                ph.enter_context(tc.tile_pool(name="psB" + sfx, bufs=3,
                                              space="PSUM")),
                ph.enter_context(tc.tile_pool(name="psS" + sfx, bufs=1,
                                              space="PSUM")),
            )

        # ===== phase B1: KV latent down-proj + rmsnorm + rope -> AG =====
        ph = ExitStack()
        pools = open_proj_pools(ph, "1")
        rawp, psS = pools[1], pools[3]
        latkv = rawp.tile([128, KVTR, OWN], F32)
        for lt in range(KVTR):
            down_slab(pools, wkvaP_d, lt, latkv[:, lt, :])
        rmsnorm(pools, latkv, latkv_n, KVT, KVL)

        # ---- rope k_pe (deinterleave folded into wkv_a on host) ----
        # swap halves via permutation matmul (rows 0:32 <-> 32:64)
        kpsw = psS.tile([DR, OWN], F32, tag="kpsw", bufs=1)
        nc.tensor.matmul(kpsw[:], perm128[0:DR, 0:DR], latkv[0:DR, KVT, :],
                         start=True, stop=True)
        kpc = rawp.tile([DR, OWN], F32)
        nc.vector.tensor_tensor(out=kpc[:], in0=latkv[0:DR, KVT, :],
                                in1=cosT[0:DR, :], op=AL.mult)
        kpsw2 = rawp.tile([DR, OWN], F32)
        nc.vector.tensor_tensor(out=kpsw2[:], in0=kpsw[:],
                                in1=sinTs[0:DR, :], op=AL.mult)
        kpeR = rawp.tile([DR, OWN], BF16)
        nc.vector.tensor_tensor(out=kpeR[:], in0=kpc[:],
                                in1=kpsw2[:], op=AL.add)

        # assemble AG input: latent rows interleaved (p a), rope rows at 512+
        nc.sync.dma_start(
            agin[0:KVL, :].rearrange("(p a) t -> p a t", a=KVT), latkv_n[:])
        nc.sync.dma_start(agin[KVL:KVL + DR, :], kpeR[:])
        if sim_mode:
            nc.sync.dma_start(agkv[0:KVL + DR, :], agin[:])
        else:
            nc.gpsimd.collective_compute(
                "AllGather", AL.bypass, replica_groups=[list(range(NCORES))],
                ins=[agin.opt()], outs=[agkv.opt()])
        ph.close()

        # ===== phases D+E interleaved: V chunks and K^T heads alternate ====
        # so the 8 AG slices stream v0,kt0,v1,kt1,... and attention can
        # start as soon as (v0, kt0) land.
        ph = ExitStack()
        wkn = ph.enter_context(tc.tile_pool(name="wkn", bufs=1))
        psE = ph.enter_context(tc.tile_pool(name="psE", bufs=2, space="PSUM"))
        eve = ph.enter_context(tc.tile_pool(name="eve", bufs=2))
        agp = ph.enter_context(tc.tile_pool(name="agp", bufs=1))
        phD = ExitStack()
        wv = phD.enter_context(tc.tile_pool(name="wv", bufs=2))
        psD = phD.enter_context(tc.tile_pool(name="psD", bufs=2, space="PSUM"))
        evp = phD.enter_context(tc.tile_pool(name="evp", bufs=3))

        wkns = wkn.tile([128, KVT, 4 * DN], BF16)
        nc.sync.dma_start(
            wkns[:], wkvbnP_d.rearrange("p (a c) -> p a c", c=4 * DN))
        slabs = agp.tile([128, NCORES, KVT, OWN], BF16)
        kpeT = const.tile([128, T], BF16)

        def d_chunk(j):
            for vc2 in range(4):        # 4 chunks of 512 v-columns each
                vc = 4 * j + vc2
                wvs = wv.tile([128, KVT, CH], BF16, tag="wvs")
                nc.sync.dma_start(
                    wvs[:], wkvbvP_d[128 * vc:128 * (vc + 1), :]
                    .rearrange("p (a c) -> p a c", c=CH))
                ev = evp.tile([128, 2, CH], BF16, tag="vev")
                for tt in range(2):     # 2 token tiles of 128
                    ps = psD.tile([128, CH], F32, tag="vps")
                    for lt in range(KVT):
                        nc.tensor.matmul(
                            ps[:], latkv_n[:, lt, 128 * tt:128 * (tt + 1)],
                            wvs[:, lt, :], start=(lt == 0),
                            stop=(lt == KVT - 1))
                    nc.scalar.copy(ev[:, tt, :], ps[:])
                nc.sync.dma_start(
                    vshards[j][:, CH * vc2:CH * (vc2 + 1)]
                    .rearrange("(tt p) c -> p tt c", p=128), ev[:])
            if sim_mode:
                nc.sync.dma_start(vfulls[j][0:OWN, :], vshards[j][:])
            else:
                nc.gpsimd.collective_compute(
                    "AllGather", AL.bypass,
                    replica_groups=[list(range(NCORES))],
                    ins=[vshards[j].opt()], outs=[vfulls[j].opt()])

        def e_head(hl):
            evA = eve.tile([128, NCORES, 128], BF16, tag="ktevA")
            evB = eve.tile([128, NCORES, 128], BF16, tag="ktevB")
            for r in range(NCORES):
                ps = psE.tile([128, OWN], F32, tag="ktps")
                for lt in range(KVT):
                    nc.tensor.matmul(ps[:], wkns[:, lt, DN * hl:DN * (hl + 1)],
                                     slabs[:, r, lt, :],
                                     start=(lt == 0), stop=(lt == KVT - 1))
                # rank r owns token blocks r (cols 0:128) and 15-r (128:256)
                nc.scalar.copy(evA[:, r, :], ps[:, 0:128])
                nc.scalar.copy(evB[:, 7 - r, :], ps[:, 128:256])
            half = hl // 2
            row0 = DN * (hl % 2)
            nc.sync.dma_start(ktshards[half][row0:row0 + DN, 0:1024], evA[:])
            nc.sync.dma_start(ktshards[half][row0:row0 + DN, 1024:2048], evB[:])
            if hl % 2 == 1:
                if sim_mode:
                    nc.sync.dma_start(ktfulls[half][0:2 * DN, :],
                                      ktshards[half][:])
                else:
                    nc.gpsimd.collective_compute(
                        "AllGather", AL.bypass,
                        replica_groups=[list(range(NCORES))],
                        ins=[ktshards[half].opt()], outs=[ktfulls[half].opt()])

        for i in range(2):
            d_chunk(i)
        phD.close()

        # agkv-dependent slab loads go on the scalar queue: they must wait
        # for the latent AllGather, and on the sync/gpsimd queues they would
        # head-of-line block the weight streams / V collectives behind them.
        # (all 8 latent slabs resident: 16 KB/partition)
        for r in range(NCORES):
            nc.scalar.dma_start(
                slabs[:, r, :, :], agkv[(KVL + DR) * r:(KVL + DR) * r + KVL, :]
                .rearrange("(p a) t -> p a t", a=KVT))

        # q latent down-proj between D and E: E's inputs need the latent
        # AllGather, so E compute would stall the in-order tensor queue if
        # emitted first; B2 fills that window.
        ph2 = ExitStack()
        pools = open_proj_pools(ph2, "2")
        latq = pools[1].tile([128, QLT, OWN], F32)
        for lt in range(QLT):
            down_slab(pools, wqaP_d, lt, latq[:, lt, :])
        rmsnorm(pools, latq, latq_n, QLT, QL)
        ph2.close()

        # k_pe^T assembly (token-ordered; duplicated on rows 64:128 so odd
        # heads of a pair can use base partition 64); scalar queue is idle
        # here and these reads must wait on the latent AllGather anyway.
        for b in range(16):
            rb = min(b, 15 - b)
            colsl = slice(0, 128) if b < 8 else slice(128, 256)
            src_ap = agkv[(KVL + DR) * rb + KVL:(KVL + DR) * rb + KVL + DR, colsl]
            nc.scalar.dma_start(kpeT[0:DR, 128 * b:128 * (b + 1)], src_ap)
            nc.scalar.dma_start(kpeT[DR:128, 128 * b:128 * (b + 1)], src_ap)

        for i in range(4):
            e_head(i)
        ph.close()

        # ===== phase F: Q up-projection + rope (head pairs) ===========
        # pair slab columns per lt: [nope_even(128) | nope_odd(128) |
        # rope_even(64) | rope_odd(64)]; rope halves share one stationary.
        qp_pool = st.enter_context(tc.tile_pool(name="qp", bufs=1))
        qTn = qp_pool.tile([128, H, OWN], BF16)
        qTp = qp_pool.tile([128, H // 2, OWN], BF16)
        attnB = qp_pool.tile([128, H, OWN], BF16)
        recips = qp_pool.tile([128, H, 2], F32)

        ph = ExitStack()
        wqb = ph.enter_context(tc.tile_pool(name="wqb", bufs=3))
        psF = ph.enter_context(tc.tile_pool(name="psF", bufs=2, space="PSUM"))
        rp = ph.enter_context(tc.tile_pool(name="rp", bufs=3))

        for pr in range(H // 2):
            ws = wqb.tile([128, QLT, 384], BF16, tag="wqbs")
            nc.sync.dma_start(
                ws[:], wqbP_d[128 * pr:128 * (pr + 1), :]
                .rearrange("p (a c) -> p a c", c=384))
            psnE = psF.tile([128, OWN], F32, tag="qnE")
            psnO = psF.tile([128, OWN], F32, tag="qnO")
            psp = psF.tile([128, OWN], F32, tag="qpp")
            for lt in range(QLT):
                nc.tensor.matmul(psnE[:], ws[:, lt, 0:128], latq_n[:, lt, :],
                                 start=(lt == 0), stop=(lt == QLT - 1))
            for lt in range(QLT):
                nc.tensor.matmul(psnO[:], ws[:, lt, 128:256], latq_n[:, lt, :],
                                 start=(lt == 0), stop=(lt == QLT - 1))
            for lt in range(QLT):
                nc.tensor.matmul(psp[:], ws[:, lt, 256:384],
                                 latq_n[:, lt, :],
                                 start=(lt == 0), stop=(lt == QLT - 1))
            nc.scalar.copy(qTn[:, 2 * pr, :], psnE[:])
            nc.scalar.copy(qTn[:, 2 * pr + 1, :], psnO[:])
            praw = rp.tile([128, OWN], F32, tag="praw")
            nc.scalar.copy(praw[:], psp[:])
            psw = psF.tile([128, OWN], F32, tag="psw")
            nc.tensor.matmul(psw[:], perm128[:], praw[:], start=True, stop=True)
            pc = rp.tile([128, OWN], F32, tag="pc")
            nc.vector.tensor_tensor(out=pc[:], in0=praw[:],
                                    in1=cosT[:], op=AL.mult)
            psw2 = rp.tile([128, OWN], F32, tag="psw2")
            nc.vector.tensor_tensor(out=psw2[:], in0=psw[:],
                                    in1=sinTs[:], op=AL.mult)
            nc.vector.tensor_tensor(out=qTp[:, pr, :], in0=pc[:],
                                    in1=psw2[:], op=AL.add)
        ph.close()

        # wo slabs prefetch on the (otherwise idle) gpsimd DMA queue while
        # attention runs; the pool lives outside G's pools so its SBUF does
        # not alias G tiles.
        phw = ExitStack()
        wop = phw.enter_context(tc.tile_pool(name="wop", bufs=8))
        wo_tiles = {}

        def wo_load(oc):
            ws = wop.tile([128, H, 128], BF16, tag="wos")
            nc.gpsimd.dma_start(
                ws[:], woP_d[128 * oc:128 * (oc + 1), :]
                .rearrange("p (a c) -> p a c", c=128))
            wo_tiles[oc] = ws

        # =========== phase G: attention (single-pass softmax, no max) ======
        # Software-pipelined: head h+1's scores are issued before head h's
        # PV block so the PE stream never waits on the softmax chain.
        ph = ExitStack()
        ap = ph.enter_context(tc.tile_pool(name="ap", bufs=2))
        sp_pool = ph.enter_context(tc.tile_pool(name="spp", bufs=2))
        stt = ph.enter_context(tc.tile_pool(name="stt", bufs=2))
        abuf = ph.enter_context(tc.tile_pool(name="abuf", bufs=2))
        psG = ph.enter_context(tc.tile_pool(name="psG", bufs=2, space="PSUM"))
        psT = ph.enter_context(tc.tile_pool(name="psT", bufs=2, space="PSUM"))
        psV = ph.enter_context(tc.tile_pool(name="psV", bufs=2, space="PSUM"))
        psN = ph.enter_context(tc.tile_pool(name="psN", bufs=1, space="PSUM"))

        # unit static params: (qsel, koff) ; u4 handled as two variants
        UQ = [1, 1, 1, 0]          # u0-2: tile B (q cols 128:256), u3: tile A
        UK = [0, CH, 2 * CH, 0]
        U4A = (0, CH)              # qsel A, keys 512:1024
        U4B = (1, 3 * CH)          # qsel B, keys 1536:2048
        # V slot indices (rank-major AG layout) per unit
        UVB = [[_vslot(j) for j in blocks] for blocks in
               [[0, 1, 2, 3], [4, 5, 6, 7], [8, 9, 10, 11], [0, 1, 2, 3]]]
        U4AV = [_vslot(j) for j in [4, 5, 6, 7]]
        U4BV = [_vslot(j) for j in [12, 13, 14, 15]]

        Pbs = {}
        sumus = {}

        def g_pass1(h):
            kt = ap.tile([128, T], BF16, tag="kt")
            row0 = 2 * DN * (h // 4) + DN * (h % 2)
            nc.sync.dma_start(
                kt[:], ktfulls[(h % 4) // 2][row0:row0 + DN, :])
            Pb = sp_pool.tile([128, NU, CH], BF16, tag="Pb")
            sumu = stt.tile([128, NU], F32, tag="sumu")
            Pbs[h] = Pb
            sumus[h] = sumu
            pb = 64 * (h % 2)
            pr = h // 2

            def scores(qsel, koff, psname):
                ps = psG.tile([128, CH], F32, tag="sps", name=psname)
                q0 = 128 * qsel
                nc.tensor.matmul(ps[:], qTn[:, h, q0:q0 + 128],
                                 kt[:, koff:koff + CH], start=True, stop=False)
                nc.tensor.matmul(ps[:], qTp[pb:pb + DR, pr, q0:q0 + 128],
                                 kpeT[pb:pb + DR, koff:koff + CH],
                                 start=False, stop=True)
                return ps

            def expu(u, src):
                nc.scalar.activation(Pb[:, u, :], src, AF.Exp,
                                     scale=SCALING,
                                     accum_out=sumu[:, u:u + 1])

            # u0, u1: fully causal-visible; exp straight from PSUM
            for u in (0, 1):
                ps = scores(UQ[u], UK[u], f"s{h}_{u}")
                expu(u, ps[:])
            # u2, u3: mask-add then exp
            for u in (2, 3):
                ps = scores(UQ[u], UK[u], f"s{h}_{u}")
                S = sp_pool.tile([128, CH], F32, tag="Smask", bufs=3)
                nc.vector.tensor_tensor(out=S[:], in0=ps[:],
                                        in1=mask5[:, u, :], op=AL.add)
                expu(u, S[:])
            # u4: select variant, mask, exp
            ps4a = scores(*U4A, f"s{h}_4a")
            ps4b = scores(*U4B, f"s{h}_4b")
            S4 = sp_pool.tile([128, CH], F32, tag="Smask", bufs=3)
            nc.vector.scalar_tensor_tensor(
                out=S4[:], in0=ps4a[:], scalar=selAu4[:],
                in1=mask5[:, 4, :], op0=AL.mult, op1=AL.add)
            nc.vector.scalar_tensor_tensor(
                out=S4[:], in0=ps4b[:], scalar=selBu4[:],
                in1=S4[:], op0=AL.mult, op1=AL.add)
            expu(4, S4[:])

            # per-tile softmax denominators (reciprocals, per token row)
            sumu2 = stt.tile([128, 2], F32, tag="sumu2")
            tA = stt.tile([128, NU], F32, tag="tA")
            nc.vector.tensor_tensor(out=tA[:], in0=sumu[:], in1=selA01[:],
                                    op=AL.mult)
            nc.vector.tensor_reduce(sumu2[:, 0:1], tA[:],
                                    axis=mybir.AxisListType.X, op=AL.add)
            tB = stt.tile([128, NU], F32, tag="tB")
            nc.vector.tensor_tensor(out=tB[:], in0=sumu[:], in1=selB01[:],
                                    op=AL.mult)
            nc.vector.tensor_reduce(sumu2[:, 1:2], tB[:],
                                    axis=mybir.AxisListType.X, op=AL.add)
            nc.vector.reciprocal(recips[:, h, :], sumu2[:])

        def g_pass2(h):
            vh = ap.tile([128, 16, DV], BF16, tag="vh")
            nc.sync.dma_start(
                vh[:], vfulls[h // 16].rearrange("(b p) d -> p b d", p=128)
                [:, :, DV * (h % 16):DV * (h % 16 + 1)])
            Pb = Pbs.pop(h)
            sumus.pop(h)

            # transpose all 4 key-blocks of a unit into one PSUM bank,
            # evict with a single 512-wide copy
            def ptT_unit(u, engine):
                tp = psT.tile([128, 4, 128], BF16, tag="tp")
                for kb in range(4):
                    nc.tensor.transpose(tp[:, kb, :],
                                        Pb[:, u, 128 * kb:128 * (kb + 1)],
                                        ident[:])
                ptT = stt.tile([128, 4, 128], BF16, tag="ptT", bufs=3)
                if engine == 0:
                    nc.vector.tensor_copy(ptT[:], tp[:])
                else:
                    nc.scalar.copy(ptT[:], tp[:])
                return ptT

            att = abuf.tile([128, OWN], F32, tag="att")

            # units 0-2 accumulate into one PSUM tile (B half)
            psBh = psV.tile([128, 128], F32, tag="pvps", name=f"pvB{h}")
            for u in (0, 1, 2):
                ptT = ptT_unit(u, u % 2)
                for kb in range(4):
                    nc.tensor.matmul(psBh[:], vh[:, UVB[u][kb], :],
                                     ptT[:, kb, :],
                                     start=(u == 0 and kb == 0),
                                     stop=(u == 2 and kb == 3))
            nc.vector.tensor_copy(att[:, 128:256], psBh[:])
            # unit 3 (A half)
            psAh = psV.tile([128, 128], F32, tag="pvps", name=f"pvA{h}")
            ptT = ptT_unit(3, 1)
            for kb in range(4):
                nc.tensor.matmul(psAh[:], vh[:, UVB[3][kb], :], ptT[:, kb, :],
                                 start=(kb == 0), stop=(kb == 3))
            nc.vector.tensor_copy(att[:, 0:128], psAh[:])
            # u4: same P^T, two V variants, masked adds
            ps4av = psV.tile([128, 128], F32, tag="pvps", name=f"pv{h}_4a")
            ps4bv = psV.tile([128, 128], F32, tag="pvps", name=f"pv{h}_4b")
            ptT = ptT_unit(4, 0)
            for kb in range(4):
                nc.tensor.matmul(ps4av[:], vh[:, U4AV[kb], :], ptT[:, kb, :],
                                 start=(kb == 0), stop=(kb == 3))
                nc.tensor.matmul(ps4bv[:], vh[:, U4BV[kb], :], ptT[:, kb, :],
                                 start=(kb == 0), stop=(kb == 3))
            nc.vector.scalar_tensor_tensor(
                out=att[:, 0:128], in0=ps4av[:], scalar=selAu4[:],
                in1=att[:, 0:128], op0=AL.mult, op1=AL.add)
            nc.vector.scalar_tensor_tensor(
                out=att[:, 128:256], in0=ps4bv[:], scalar=selBu4[:],
                in1=att[:, 128:256], op0=AL.mult, op1=AL.add)

            # normalize: transpose per-token reciprocals into one row,
            # broadcast to all partitions, multiply
            rT = psN.tile([1, OWN], F32, tag="rT", bufs=1)
            nc.tensor.matmul(rT[0:1, 0:128], recips[:, h, 0:1], identf[:],
                             start=True, stop=True)
            nc.tensor.matmul(rT[0:1, 128:256], recips[:, h, 1:2], identf[:],
                             start=True, stop=True)
            recT = stt.tile([1, OWN], F32, tag="recT", bufs=2)
            nc.scalar.copy(recT[:], rT[:])
            nb = psN.tile([128, OWN], F32, tag="nb", bufs=1)
            nc.tensor.matmul(nb[:], onesrow[:], recT[:], start=True, stop=True)
            nc.vector.tensor_tensor(out=attnB[:, h, :], in0=nb[:],
                                    in1=att[:], op=AL.mult)

        # process heads in AG-arrival order: K^T slice i carries heads 4r+i
        HORDER = [4 * r + i for i in range(4) for r in range(NCORES)]
        for oc in range(8):
            wo_load(oc)
        g_pass1(HORDER[0])
        for idx, h in enumerate(HORDER):
            if idx + 1 < H:
                g_pass1(HORDER[idx + 1])
            g_pass2(h)
        ph.close()

        # =========== phase H: out projection ===========
        ph = ExitStack()
        psH = ph.enter_context(tc.tile_pool(name="psH", bufs=4, space="PSUM"))
        oev = ph.enter_context(tc.tile_pool(name="oev", bufs=3))
        for oc in range(HT):
            if oc + 8 < HT:
                wo_load(oc + 8)
            ws = wo_tiles.pop(oc)
            ps = psH.tile([128, OWN], F32, tag="ops")
            for ct in range(H):
                nc.tensor.matmul(ps[:], ws[:, ct, :], attnB[:, ct, :],
                                 start=(ct == 0), stop=(ct == H - 1))
            ev = oev.tile([128, OWN], F32, tag="oev")
            nc.scalar.copy(ev[:], ps[:])
            nc.sync.dma_start(outT_d[128 * oc:128 * (oc + 1), :], ev[:])
        ph.close()
        phw.close()
        st.close()

    nc.finalize()
    legalize_sync_waits(nc)
    return nc


_DEINT = np.array([2 * r if r < 32 else 2 * r - 63 for r in range(DR)])


def _slabs(w, rows_per_slab=128):
    """[R, C] -> [R/128 * 128, (R_tiles) * C] contiguous slab layout:
    out[s*128 + p, a*C + c] = w[a*128 + p, s*C_slab + c] per slab s of cols."""
    raise NotImplementedError


def _pa(w):
    """[A*128, C] -> [128, A*C]: row-block-major to partition-major."""
    A = w.shape[0] // 128
    return w.reshape(A, 128, w.shape[1]).transpose(1, 0, 2).reshape(128, -1)


def _host_prep(inputs):
    f32 = np.float32
    bf16 = ml_dtypes.bfloat16
    hs = np.asarray(inputs["hidden_states"], f32)
    cos = np.asarray(inputs["cos"], f32).reshape(T, DR)
    sin = np.asarray(inputs["sin"], f32).reshape(T, DR)
    wq_a = np.asarray(inputs["wq_a"], f32)
    q_ln = np.asarray(inputs["q_a_ln_w"], f32)
    wq_b = np.asarray(inputs["wq_b"], f32)
    wkv_a = np.asarray(inputs["wkv_a"], f32)
    kv_ln = np.asarray(inputs["kv_a_ln_w"], f32)
    wkv_b = np.asarray(inputs["wkv_b"], f32)
    wo = np.asarray(inputs["wo"], f32)

    # fold ln weights into up-projections
    wq_b = wq_b * q_ln[:, None]
    wkv_b = wkv_b * kv_ln[:, None]

    # deinterleave fold: q_pe columns of wq_b, k_pe columns of wkv_a
    wqbp = wq_b.copy()
    for h in range(H):
        pe = wq_b[:, h * DQK + DN:h * DQK + DQK]
        wqbp[:, h * DQK + DN:h * DQK + DQK] = pe[:, _DEINT]
    wkvap = np.zeros((HID, KVTR * 128), f32)
    wkvap[:, 0:KVL] = wkv_a[:, 0:KVL]
    wkvap[:, KVL:KVL + DR] = wkv_a[:, KVL:][:, _DEINT]

    # split wkv_b into nope / v column groups (head-major)
    wkvbn = np.concatenate(
        [wkv_b[:, h * 256:h * 256 + DN] for h in range(H)], axis=1)  # [512,4096]
    wkvbv = np.concatenate(
        [wkv_b[:, h * 256 + DN:h * 256 + 256] for h in range(H)], axis=1)

    # ---- contiguous slab layouts (one DMA per slab, contiguous rows) ----
    # wqaP: slab lt rows -> [128, 40*128] from wq_a cols [128lt:128lt+128]
    wqaP = np.concatenate(
        [_pa(wq_a[:, 128 * lt:128 * (lt + 1)]) for lt in range(QLT)], axis=0)
    wkvaP = np.concatenate(
        [_pa(wkvap[:, 128 * lt:128 * (lt + 1)]) for lt in range(KVTR)], axis=0)
    # head-pair slabs: [nope_even | nope_odd | rope_even | rope_odd]
    wqbP = np.concatenate(
        [_pa(np.concatenate([
            wqbp[:, DQK * (2 * pr):DQK * (2 * pr) + DN],
            wqbp[:, DQK * (2 * pr + 1):DQK * (2 * pr + 1) + DN],
            wqbp[:, DQK * (2 * pr) + DN:DQK * (2 * pr + 1)],
            wqbp[:, DQK * (2 * pr + 1) + DN:DQK * (2 * pr + 2)],
        ], axis=1)) for pr in range(H // 2)], axis=0)
    wkvbvP = np.concatenate(
        [_pa(wkvbv[:, CH * vc:CH * (vc + 1)]) for vc in range(8)], axis=0)
    woP = np.concatenate(
        [_pa(wo[:, 128 * oc:128 * (oc + 1)]) for oc in range(HT)], axis=0)

    cosTf = np.ascontiguousarray(cos.T)          # [64, 2048]
    sinTf = np.ascontiguousarray(sin.T)
    sinTs_f = sinTf.copy()
    sinTs_f[0:32] = -sinTf[0:32]

    ident = np.eye(128, dtype=bf16)
    perm128 = np.zeros((128, 128), f32)
    for o in range(DR):
        perm128[(o + 32) % DR, o] = 1.0
        perm128[DR + (o + 32) % DR, DR + o] = 1.0
    ones128 = np.ones((128, 1), f32)
    onesrow = np.ones((1, 128), f32)

    wqaP_bf = wqaP.astype(bf16)
    wqbP_bf = wqbP.astype(bf16)
    wkvaP_bf = wkvaP.astype(bf16)
    wkvbvP_bf = wkvbvP.astype(bf16)
    woP_bf = woP.astype(bf16)
    wkvbn_bf = wkvbn.astype(bf16)

    in_maps = []
    for c in range(NCORES):
        bA, bB = c, 15 - c
        own = np.r_[np.arange(128 * bA, 128 * bA + 128),
                    np.arange(128 * bB, 128 * bB + 128)]
        units = _unit_descs(c)
        mask5 = np.zeros((128, NU, CH), f32)
        selA01 = np.zeros((128, NU), f32)
        selB01 = np.zeros((128, NU), f32)
        for u, (tl, j) in enumerate(units):
            b = bA if tl == "A" else bB
            qtok = 128 * b + np.arange(128)[:, None]
            ktok = CH * j + np.arange(CH)[None, :]
            mask5[:, u, :] = np.where(ktok <= qtok, 0.0, NEG)
            (selA01 if tl == "A" else selB01)[:, u] = 1.0
        selAu4 = np.full((128, 1), 1.0 if c >= 4 else 0.0, f32)
        selBu4 = np.full((128, 1), 0.0 if c >= 4 else 1.0, f32)

        hidT = np.ascontiguousarray(hs[own].T)   # [5120, 256]
        hidP = _pa(hidT)                         # [128, 40*256]

        # this core's 4 nope heads: [512, 512] -> [128, 4*512]
        wkvbnP = _pa(wkvbn_bf[:, 4 * DN * c:4 * DN * (c + 1)].astype(f32)).astype(bf16)

        in_maps.append({
            "hidP": hidP.astype(bf16),
            "cosT": np.ascontiguousarray(np.vstack([cosTf[:, own]] * 2)),
            "sinTs": np.ascontiguousarray(np.vstack([sinTs_f[:, own]] * 2)),
            "wqaP": wqaP_bf,
            "wqbP": wqbP_bf,
            "wkvaP": wkvaP_bf,
            "wkvbnP": wkvbnP,
            "wkvbvP": wkvbvP_bf,
            "woP": woP_bf,
            "mask5": mask5,
            "selA01": selA01, "selB01": selB01,
            "selAu4": selAu4, "selBu4": selBu4,
            "ident": ident, "perm128": perm128,
            "ones128": ones128, "onesrow": onesrow,
        })
    return in_maps


_NC_CACHE = None


def _get_nc():
    global _NC_CACHE
    if _NC_CACHE is None:
        _NC_CACHE = build_bass()
    return _NC_CACHE


def run(inputs, trace=False):
    nc = _get_nc()
    in_maps = _host_prep(inputs)
    res = run_bass_kernel_spmd(nc, in_maps, list(range(NCORES)), trace=trace)
    out = np.empty((T, HID), np.float32)
    for c in range(NCORES):
        oT = res.results[c]["outT"]
        out[128 * c:128 * (c + 1)] = oT[:, 0:128].T
        out[128 * (15 - c):128 * (16 - c)] = oT[:, 128:256].T
    return out, res


def kernel(**inputs):
    out, _ = run(inputs, trace=False)
    return out
